# revision 13
# baseline (speedup 1.0000x reference)
"""ChebConv (complex, K+1=3 hops) Trainium2 kernel over 8 NeuronCores.

Sharding: 1D node partition on destination rows (6250 rows/core), full X
replicated; each core processes exactly the edges targeting its rows, so no
collectives are needed.

Per core, edges are sorted by (batch of 126 dest rows, col half, group of 14
dest rows) and packed into 128-edge blocks with NO per-group alignment; only
the (batch, half) bucket is padded to a block multiple, and the padding uses
idx=-1 with a per-core valid-count register so it transfers nothing.  One
dma_gather per (batch, half) -> 100 calls/core (vs 596 in the per-group
scheme), each moving ~2MB, so the ~1us SWDGE fixed cost amortizes away and
the gather runs at the HBM random-read rate on the minimal 204.8MB/core.

A block may span two adjacent 14-row groups; each (block, group) pair gets
its own one-hot V matmul, with rows outside the target group never matching
the 0..13 one-hot columns (masked to zero).  Per batch of 9 groups the four
spmm partial aggregates land feature-major in PSUM ([128 feat, 6 slots x 14
rows] x 4 quarters per group), then 24 bf16 matmuls with signed weight
tiles (bf16) contract features and produce row-major outputs directly.
"""
import os
import sys
sys.path.insert(0, '/opt/trn_rl_repo')

import numpy as np
import ml_dtypes

ABLATE = os.environ.get("ABLATE", "")   # '', 'nogather', 'nocompute'

N = 50000
E = 1_600_000
K1 = 3
C = 256
CORES = 8
RPC = N // CORES            # 6250 rows per core
GR = 14                     # rows per group
GPB = 9                     # groups per batch
MCOLS = 6 * GR              # 84 one-hot columns per (block, group) pair
ROWS_PB = GR * GPB          # 126
NB = -(-RPC // ROWS_PB)     # 50
NGRP = NB * GPB             # 450
REAL_GRP = -(-RPC // GR)    # 447
HALF = 32768
NQ = 4                      # SWDGE queues
GT_BUFS = 2                 # gather-tile ring depth
USE_COUNT_REG = True        # per-core valid-count register + idx=-1 padding
                            # (works on HW only WITHOUT value_load's runtime
                            #  bounds assert -- min_val/max_val must be None)
GMAX = 8                    # max blocks per dma_gather call: the SWDGE ring
                            # holds 1024 descriptors; >= 12 blocks faults


def _bf16(x):
    return x.astype(ml_dtypes.bfloat16)


def _preprocess(rows, cols, Lr, Li, weight, bias):
    rows = np.asarray(rows).astype(np.int64)
    cols = np.asarray(cols).astype(np.int64)
    core = rows // RPC
    rloc = rows - core * RPC
    gg = rloc // GR
    bt_e = gg // GPB
    gl_e = gg - bt_e * GPB
    colh = (cols >= HALF).astype(np.int64)

    C6 = np.empty((E, 6), np.float32)
    C6[:, 0:3] = np.asarray(Lr).T
    C6[:, 3:6] = np.asarray(Li).T

    # order all edges by (core, batch, col-half, group); stable so layout is
    # deterministic
    fkey = ((core * NB + bt_e) * 2 + colh) * GPB + gl_e
    order = np.argsort(fkey, kind="stable")
    cols_s = cols[order]
    C6_s = C6[order]
    rloc_s = rloc[order]
    nfine = CORES * NB * 2 * GPB
    fb = np.searchsorted(fkey[order], np.arange(nfine + 1))
    cntg = (fb[1:] - fb[:-1]).reshape(CORES, NB, 2, GPB)
    cntb = cntg.sum(axis=3)                               # [CORES, NB, 2]
    nbh = np.maximum(-(-cntb.max(axis=0) // 128), 1)      # [NB, 2]

    # shared program structure: per (batch, half) one gather call covering
    # nbh blocks; per block the union (over cores) of groups present
    calls = []          # dicts: bt, h, b0, nbh, p0, prs, ci
    pairs_by_batch = [[[] for _ in range(GPB)] for _ in range(NB)]
    b0 = p0 = ci = 0
    for bt in range(NB):
        nb0 = int(nbh[bt, 0])
        for h in range(2):
            nbn = int(nbh[bt, h])
            present = np.zeros((nbn, GPB), bool)
            for c in range(CORES):
                cum = 0
                for gl in range(GPB):
                    n = int(cntg[c, bt, h, gl])
                    if n:
                        present[cum // 128:(cum + n - 1) // 128 + 1, gl] = True
                        cum += n
            prs = [(gl, blk) for gl in range(GPB) for blk in range(nbn)
                   if present[blk, gl]]
            subs = []
            for k0 in range(0, nbn, GMAX):
                subs.append((ci, k0, min(GMAX, nbn - k0)))
                ci += 1
            calls.append(dict(bt=bt, h=h, b0=b0, nbh=nbn, p0=p0, prs=prs,
                              subs=subs))
            b0 += nbn
            p0 += len(prs)
        voff = 0
        for h in range(2):
            cl = calls[bt * 2 + h]
            for pi, (gl, blk) in enumerate(cl["prs"]):
                blk_local = (nb0 if h == 1 else 0) + blk
                pairs_by_batch[bt][gl].append((blk_local, voff + pi))
            voff += len(cl["prs"])
    tot_blk, npair, ncall = b0, p0, ci
    nbt_max = int(nbh.sum(axis=1).max())
    npbt_max = max(len(calls[2 * b]["prs"]) + len(calls[2 * b + 1]["prs"])
                   for b in range(NB))
    npc_max = max(len(cl["prs"]) for cl in calls)

    per_core = []
    for c in range(CORES):
        idx16 = np.full(tot_blk * 128, -1, np.int16)
        c6p = np.zeros((128, npair * 6), np.float32)
        jlp = np.full((128, npair), -2.0, np.float32)
        cnt32 = np.zeros(ncall, np.int32)
        for cl in calls:
            base = ((c * NB + cl["bt"]) * 2 + cl["h"]) * GPB
            lo, hi = fb[base], fb[base + GPB]
            ne = hi - lo
            if not USE_COUNT_REG:
                idx16[cl["b0"] * 128:(cl["b0"] + cl["nbh"]) * 128] = 0
            if ne:
                idx16[cl["b0"] * 128:cl["b0"] * 128 + ne] = \
                    (cols_s[lo:hi] - cl["h"] * HALF).astype(np.int16)
            for ci_s, k0, nb_s in cl["subs"]:
                vc = min(max(ne - k0 * 128, 0), nb_s * 128)
                if vc == 0:
                    idx16[(cl["b0"] + k0) * 128] = 0
                    vc = 1
                cnt32[ci_s] = vc
            ecols = C6_s[lo:hi]
            erl = rloc_s[lo:hi]
            gbase = cl["bt"] * GPB
            for pi, (gl, blk) in enumerate(cl["prs"]):
                a, b = blk * 128, min(blk * 128 + 128, ne)
                if a >= b:
                    continue
                P = cl["p0"] + pi
                c6p[0:b - a, P * 6:P * 6 + 6] = ecols[a:b]
                jlp[0:b - a, P] = np.clip(erl[a:b] - (gbase + gl) * GR, -2, 20)
        # wrap idxs: idx i lives at [i%16, i//16]; replicate to 128 partitions
        idxw = np.tile(idx16.reshape(-1, 16).T, (8, 1))
        per_core.append(dict(
            idx=np.ascontiguousarray(idxw),
            c6=np.ascontiguousarray(_bf16(c6p)),
            jl=np.ascontiguousarray(_bf16(jlp)),
            cnt=np.ascontiguousarray(np.tile(cnt32, (128, 1))),
        ))

    # weight tiles [12][128, 256] f32: 0..5 = +W[k][fh], 6..11 = -W[k][fh]
    weight = np.asarray(weight, np.float32)
    wt = np.empty((12, 128, C), np.float32)
    for fh in range(2):
        for k in range(K1):
            wt[fh * 3 + k] = weight[k][fh * 128:(fh + 1) * 128]
            wt[6 + fh * 3 + k] = -weight[k][fh * 128:(fh + 1) * 128]
    wsb = np.ascontiguousarray(_bf16(wt.transpose(1, 0, 2).reshape(128, 12 * C)))

    biasr = np.ascontiguousarray(np.tile(np.asarray(bias, np.float32), (128, 1)))
    # 14-wide one-hot pattern per pair position: column j of pair r is j
    mdw = np.ascontiguousarray(_bf16(np.tile(
        (np.arange(npc_max * GR) % GR).astype(np.float32), (128, 1))))

    return dict(tot_blk=tot_blk, npair=npair, ncall=ncall, calls=calls,
                pairs_by_batch=pairs_by_batch, nbt_max=nbt_max,
                npbt_max=npbt_max, npc_max=npc_max, per_core=per_core,
                wsb=wsb, biasr=biasr, mdw=mdw)


def _final_mm_list():
    """(target, q, s, wtile): target 0=real 1=imag; q = P quarter; s = slot."""
    mms = []
    for tgt in range(2):
        for fh in range(2):
            for k in range(K1):
                if tgt == 0:
                    mms.append((0, fh, k, fh * 3 + k))               # +W  Lr@Xr
                    mms.append((0, 2 + fh, 3 + k, 6 + fh * 3 + k))   # -W  Li@Xi
                else:
                    mms.append((1, fh, 3 + k, fh * 3 + k))           # +W  Li@Xr
                    mms.append((1, 2 + fh, k, fh * 3 + k))           # +W  Lr@Xi
    return mms


def _build(nc, prep, repeat=1):
    import concourse.mybir as mybir
    from concourse.tile import TileContext

    f32 = mybir.dt.float32
    bf16 = mybir.dt.bfloat16
    i16 = mybir.dt.int16
    i32 = mybir.dt.int32
    tot_blk = prep["tot_blk"]
    npair = prep["npair"]
    ncall = prep["ncall"]
    calls = prep["calls"]
    pairs_by_batch = prep["pairs_by_batch"]
    nbt_max = prep["nbt_max"]
    npbt_max = prep["npbt_max"]
    npc_max = prep["npc_max"]

    xcat = nc.dram_tensor("xcat", [N, 512], bf16, kind="ExternalInput")
    idx_d = nc.dram_tensor("idx", [128, tot_blk * 8], i16, kind="ExternalInput")
    c6_d = nc.dram_tensor("c6", [128, npair * 6], bf16, kind="ExternalInput")
    jl_d = nc.dram_tensor("jl", [128, npair], bf16, kind="ExternalInput")
    cnt_d = nc.dram_tensor("cnt", [128, ncall], i32, kind="ExternalInput")
    w_d = nc.dram_tensor("wt", [128, 12 * C], bf16, kind="ExternalInput")
    bias_d = nc.dram_tensor("biasr", [128, C], f32, kind="ExternalInput")
    md_d = nc.dram_tensor("mdw", [128, npc_max * GR], bf16,
                          kind="ExternalInput")
    or_d = nc.dram_tensor("out_r", [NB * ROWS_PB, C], bf16, kind="ExternalOutput")
    oi_d = nc.dram_tensor("out_i", [NB * ROWS_PB, C], bf16, kind="ExternalOutput")

    mms = _final_mm_list()

    import contextlib

    with TileContext(nc) as tc:
        with tc.tile_pool(name="const", bufs=1) as cpool, \
             tc.tile_pool(name="g", bufs=GT_BUFS) as gpool, \
             tc.tile_pool(name="v", bufs=2) as vpool, \
             tc.tile_pool(name="ohp", bufs=2) as ohpool, \
             tc.tile_pool(name="pb", bufs=2) as pbpool, \
             tc.tile_pool(name="os", bufs=3) as ospool, \
             tc.tile_pool(name="ps", bufs=3, space="PSUM") as pspool, \
             tc.tile_pool(name="po", bufs=2, space="PSUM") as popool:

            idx_t = cpool.tile([128, tot_blk * 8], i16)
            c6_t = cpool.tile([128, npair * 6], bf16)
            jl_t = cpool.tile([128, npair], bf16)
            cnt_t = cpool.tile([128, ncall], i32)
            w_t = cpool.tile([128, 12 * C], bf16)
            bias_t = cpool.tile([128, C], f32)
            md_t = cpool.tile([128, npc_max * GR], bf16)
            for dst, src in [(idx_t, idx_d), (c6_t, c6_d), (jl_t, jl_d),
                             (cnt_t, cnt_d), (w_t, w_d), (bias_t, bias_d),
                             (md_t, md_d)]:
                nc.sync.dma_start(dst[:], src[:])

            rep_cm = tc.For_i(0, repeat, 1) if repeat > 1 else contextlib.nullcontext()
            with rep_cm:
              qn = 0
              for bt in range(NB):
                  c0, c1 = calls[2 * bt], calls[2 * bt + 1]
                  nb0 = c0["nbh"]
                  gt = gpool.tile([128, nbt_max * 512], bf16, tag="g")
                  if USE_COUNT_REG and bt < GT_BUFS:
                      # first touch of each ring buffer: idx=-1 pad lanes are
                      # never written by the gather, so clear once (later
                      # batches inherit finite stale data, zeroed by V)
                      nc.vector.memset(gt[:], 0.0)
                  for j, cl in enumerate((c0, c1)):
                      src = xcat[:] if cl["h"] == 0 else xcat[HALF:, :]
                      off = 0 if j == 0 else nb0
                      for ci_s, k0, nb_s in cl["subs"]:
                          if USE_COUNT_REG:
                              reg = nc.gpsimd.value_load(
                                  cnt_t[0:1, ci_s:ci_s + 1])
                          else:
                              reg = nb_s * 128
                          if ABLATE != "nogather":
                              nc.gpsimd.dma_gather(
                                  gt[:, (off + k0) * 512:(off + k0 + nb_s) * 512]
                                    .rearrange("p (b e) -> p b e", e=512),
                                  src,
                                  idx_t[:, (cl["b0"] + k0) * 8:
                                           (cl["b0"] + k0 + nb_s) * 8],
                                  nb_s * 128, reg, 512,
                                  queue_num=qn,
                              )
                              qn = (qn + 1) % NQ
                  if ABLATE == "nocompute":
                      o_r = ospool.tile([128, C], bf16, tag="or")
                      o_i = ospool.tile([128, C], bf16, tag="oi")
                      nc.scalar.copy(o_r[:ROWS_PB, :], bias_t[:ROWS_PB, :])
                      nc.scalar.copy(o_i[:ROWS_PB, :], bias_t[:ROWS_PB, :])
                      nc.sync.dma_start(or_d[bt * ROWS_PB:(bt + 1) * ROWS_PB, :],
                                        o_r[:ROWS_PB, :])
                      nc.sync.dma_start(oi_d[bt * ROWS_PB:(bt + 1) * ROWS_PB, :],
                                        o_i[:ROWS_PB, :])
                      continue
                  # one-hot V for every (block, group) pair of the batch:
                  # 14-wide one-hot (is_equal) then one broadcast multiply
                  # expands x6 slots with the edge vals
                  vt = vpool.tile([128, npbt_max * MCOLS], bf16, tag="v")
                  oh = ohpool.tile([128, npbt_max * GR], bf16, tag="oh")
                  voff = 0
                  for cl in (c0, c1):
                      np_ = len(cl["prs"])
                      if np_ == 0:
                          continue
                      ohsl = oh[:, voff * GR:(voff + np_) * GR]
                      nc.vector.tensor_tensor(
                          ohsl.rearrange("p (r j) -> p r j", j=GR),
                          md_t[:, :np_ * GR]
                              .rearrange("p (r j) -> p r j", j=GR),
                          jl_t[:, cl["p0"]:cl["p0"] + np_]
                              .unsqueeze(2).broadcast_to((128, np_, GR)),
                          mybir.AluOpType.is_equal)
                      nc.vector.tensor_tensor(
                          vt[:, voff * MCOLS:(voff + np_) * MCOLS]
                              .rearrange("p (r s j) -> p r s j", s=6, j=GR),
                          ohsl.rearrange("p (r j) -> p r j", j=GR)
                              .unsqueeze(2).broadcast_to((128, np_, 6, GR)),
                          c6_t[:, cl["p0"] * 6:(cl["p0"] + np_) * 6]
                              .rearrange("p (r s) -> p r s", s=6)
                              .unsqueeze(3).broadcast_to((128, np_, 6, GR)),
                          mybir.AluOpType.mult)
                      voff += np_
                  # per-group spmm accumulation (feature-major in PSUM)
                  pbuf = pbpool.tile([128, 24 * ROWS_PB], bf16, tag="pb")
                  pb5 = pbuf[:].rearrange("p (q s g j) -> p q s g j",
                                          q=4, s=6, g=GPB)
                  for gl in range(GPB):
                      prs = pairs_by_batch[bt][gl]
                      if not prs:
                          nc.vector.memset(pb5[:, :, :, gl, :], 0.0)
                          continue
                      p_t = pspool.tile([128, 512], f32, tag="p")
                      for q in range(4):
                          for i, (bl, vc) in enumerate(prs):
                              nc.tensor.matmul(
                                  p_t[:, q * 128:q * 128 + MCOLS],
                                  gt[:, bl * 512 + q * 128:bl * 512 + (q + 1) * 128],
                                  vt[:, vc * MCOLS:(vc + 1) * MCOLS],
                                  start=(i == 0), stop=(i == len(prs) - 1))
                      nc.scalar.copy(
                          pb5[:, :, :, gl, :],
                          p_t[:].rearrange("p (q x) -> p q x", x=128)[:, :, :MCOLS])
                  # final matmuls for this batch (f32r: 1 cycle/row at 256 cols)
                  po = popool.tile([128, 512], f32, tag="po")
                  nmm = {0: 0, 1: 0}
                  for tgt, q, s, wi in mms:
                      plane = q * 6 + s
                      nc.tensor.matmul(
                          po[:ROWS_PB, tgt * C:(tgt + 1) * C],
                          pbuf[:, plane * ROWS_PB:(plane + 1) * ROWS_PB],
                          w_t[:, wi * C:(wi + 1) * C],
                          start=(nmm[tgt] == 0), stop=(nmm[tgt] == 11))
                      nmm[tgt] += 1
                  o_r = ospool.tile([128, C], bf16, tag="or")
                  o_i = ospool.tile([128, C], bf16, tag="oi")
                  nc.vector.tensor_tensor(o_r[:ROWS_PB, :], po[:ROWS_PB, 0:C],
                                          bias_t[:ROWS_PB, :], mybir.AluOpType.add)
                  nc.vector.tensor_tensor(o_i[:ROWS_PB, :], po[:ROWS_PB, C:2 * C],
                                          bias_t[:ROWS_PB, :], mybir.AluOpType.add)
                  nc.sync.dma_start(or_d[bt * ROWS_PB:(bt + 1) * ROWS_PB, :],
                                    o_r[:ROWS_PB, :])
                  nc.sync.dma_start(oi_d[bt * ROWS_PB:(bt + 1) * ROWS_PB, :],
                                    o_i[:ROWS_PB, :])


def _make_nc(prep, repeat=1):
    import concourse.bacc as bacc
    nc = bacc.Bacc("TRN2", target_bir_lowering=False, debug=False,
                   num_swdge_queues=NQ)
    _build(nc, prep, repeat=repeat)
    nc.compile()
    return nc


def _in_maps(prep, X_real, X_imag):
    xcat = _bf16(np.concatenate(
        [np.asarray(X_real, np.float32), np.asarray(X_imag, np.float32)],
        axis=1))
    maps = []
    for c in range(CORES):
        pc = prep["per_core"][c]
        maps.append({
            "xcat": xcat, "idx": pc["idx"], "c6": pc["c6"], "jl": pc["jl"],
            "cnt": pc["cnt"], "wt": prep["wsb"], "biasr": prep["biasr"],
            "mdw": prep["mdw"],
        })
    return maps


def kernel(X_real, X_imag, L_real_vals, L_imag_vals, weight, bias, rows, cols):
    from concourse.bass_utils import run_bass_kernel_spmd

    prep = _preprocess(rows, cols, L_real_vals, L_imag_vals, weight, bias)
    nc = _make_nc(prep)
    res = run_bass_kernel_spmd(nc, _in_maps(prep, X_real, X_imag),
                               core_ids=list(range(CORES)))
    out_r = np.concatenate([res.results[c]["out_r"][:RPC] for c in range(CORES)], 0)
    out_i = np.concatenate([res.results[c]["out_i"][:RPC] for c in range(CORES)], 0)
    return out_r.astype(np.float32), out_i.astype(np.float32)


# revision 14
# speedup vs baseline: 1.2707x; 1.2707x over previous
"""ChebConv (complex, K+1=3 hops) Trainium2 kernel over 8 NeuronCores.

Sharding: 1D node partition on destination rows (6250 rows/core), full X
replicated; each core processes exactly the edges targeting its rows, so no
collectives are needed.

Per core, edges are sorted by (batch of 126 dest rows, col half, group of 14
dest rows) and packed into 128-edge blocks with NO per-group alignment; only
the (batch, half) bucket is padded to a block multiple, and the padding uses
idx=-1 with a per-core valid-count register so it transfers nothing.  One
dma_gather per (batch, half) -> 100 calls/core (vs 596 in the per-group
scheme), each moving ~2MB, so the ~1us SWDGE fixed cost amortizes away and
the gather runs at the HBM random-read rate on the minimal 204.8MB/core.

A block may span two adjacent 14-row groups; each (block, group) pair gets
its own one-hot V matmul, with rows outside the target group never matching
the 0..13 one-hot columns (masked to zero).  Per batch of 9 groups the four
spmm partial aggregates land feature-major in PSUM ([128 feat, 6 slots x 14
rows] x 4 quarters per group), then 24 bf16 matmuls with signed weight
tiles (bf16) contract features and produce row-major outputs directly.
"""
import os
import sys
sys.path.insert(0, '/opt/trn_rl_repo')

import numpy as np
import ml_dtypes

ABLATE = os.environ.get("ABLATE", "")   # '', 'nogather', 'nocompute'

N = 50000
E = 1_600_000
K1 = 3
C = 256
CORES = 8
RPC = N // CORES            # 6250 rows per core
GR = 14                     # rows per group
GPB = 9                     # groups per batch
MCOLS = 6 * GR              # 84 one-hot columns per (block, group) pair
ROWS_PB = GR * GPB          # 126
NB = -(-RPC // ROWS_PB)     # 50
NGRP = NB * GPB             # 450
REAL_GRP = -(-RPC // GR)    # 447
HALF = 32768
NQ = 4                      # SWDGE queues
GT_BUFS = 2                 # gather-tile ring depth
USE_COUNT_REG = True        # per-core valid-count register + idx=-1 padding
                            # (works on HW only WITHOUT value_load's runtime
                            #  bounds assert -- min_val/max_val must be None)
GMAX = 8                    # max blocks per dma_gather call: the SWDGE ring
                            # holds 1024 descriptors; >= 12 blocks faults


def _bf16(x):
    return x.astype(ml_dtypes.bfloat16)


def _preprocess(rows, cols, Lr, Li, weight, bias):
    rows = np.asarray(rows).astype(np.int64)
    cols = np.asarray(cols).astype(np.int64)
    core = rows // RPC
    rloc = rows - core * RPC
    gg = rloc // GR
    bt_e = gg // GPB
    gl_e = gg - bt_e * GPB
    colh = (cols >= HALF).astype(np.int64)

    C6 = np.empty((E, 6), np.float32)
    C6[:, 0:3] = np.asarray(Lr).T
    C6[:, 3:6] = np.asarray(Li).T

    # order all edges by (core, batch, col-half, group), then by col within a
    # segment so each DMA engine walks ascending HBM addresses
    fkey = ((core * NB + bt_e) * 2 + colh) * GPB + gl_e
    order = np.lexsort((cols, fkey))
    cols_s = cols[order]
    C6_s = C6[order]
    rloc_s = rloc[order]
    nfine = CORES * NB * 2 * GPB
    fb = np.searchsorted(fkey[order], np.arange(nfine + 1))
    cntg = (fb[1:] - fb[:-1]).reshape(CORES, NB, 2, GPB)
    cntb = cntg.sum(axis=3)                               # [CORES, NB, 2]
    nbh = np.maximum(-(-cntb.max(axis=0) // 128), 1)      # [NB, 2]

    # shared program structure: per (batch, half) one gather call covering
    # nbh blocks; per block the union (over cores) of groups present
    calls = []          # dicts: bt, h, b0, nbh, p0, prs, ci
    pairs_by_batch = [[[] for _ in range(GPB)] for _ in range(NB)]
    b0 = p0 = ci = 0
    for bt in range(NB):
        nb0 = int(nbh[bt, 0])
        for h in range(2):
            nbn = int(nbh[bt, h])
            present = np.zeros((nbn, GPB), bool)
            for c in range(CORES):
                cum = 0
                for gl in range(GPB):
                    n = int(cntg[c, bt, h, gl])
                    if n:
                        present[cum // 128:(cum + n - 1) // 128 + 1, gl] = True
                        cum += n
            prs = [(gl, blk) for gl in range(GPB) for blk in range(nbn)
                   if present[blk, gl]]
            subs = []
            for k0 in range(0, nbn, GMAX):
                subs.append((ci, k0, min(GMAX, nbn - k0)))
                ci += 1
            calls.append(dict(bt=bt, h=h, b0=b0, nbh=nbn, p0=p0, prs=prs,
                              subs=subs))
            b0 += nbn
            p0 += len(prs)
        voff = 0
        for h in range(2):
            cl = calls[bt * 2 + h]
            for pi, (gl, blk) in enumerate(cl["prs"]):
                blk_local = (nb0 if h == 1 else 0) + blk
                pairs_by_batch[bt][gl].append((blk_local, voff + pi))
            voff += len(cl["prs"])
    tot_blk, npair, ncall = b0, p0, ci
    nbt_max = int(nbh.sum(axis=1).max())
    npbt_max = max(len(calls[2 * b]["prs"]) + len(calls[2 * b + 1]["prs"])
                   for b in range(NB))
    npc_max = max(len(cl["prs"]) for cl in calls)

    per_core = []
    for c in range(CORES):
        idx16 = np.full(tot_blk * 128, -1, np.int16)
        c6p = np.zeros((128, npair * 6), np.float32)
        jlp = np.full((128, npair), -2.0, np.float32)
        cnt32 = np.zeros(ncall, np.int32)
        for cl in calls:
            base = ((c * NB + cl["bt"]) * 2 + cl["h"]) * GPB
            lo, hi = fb[base], fb[base + GPB]
            ne = hi - lo
            if not USE_COUNT_REG:
                idx16[cl["b0"] * 128:(cl["b0"] + cl["nbh"]) * 128] = 0
            if ne:
                idx16[cl["b0"] * 128:cl["b0"] * 128 + ne] = \
                    (cols_s[lo:hi] - cl["h"] * HALF).astype(np.int16)
            for ci_s, k0, nb_s in cl["subs"]:
                vc = min(max(ne - k0 * 128, 0), nb_s * 128)
                if vc == 0:
                    idx16[(cl["b0"] + k0) * 128] = 0
                    vc = 1
                cnt32[ci_s] = vc
            ecols = C6_s[lo:hi]
            erl = rloc_s[lo:hi]
            gbase = cl["bt"] * GPB
            for pi, (gl, blk) in enumerate(cl["prs"]):
                a, b = blk * 128, min(blk * 128 + 128, ne)
                if a >= b:
                    continue
                P = cl["p0"] + pi
                c6p[0:b - a, P * 6:P * 6 + 6] = ecols[a:b]
                jlp[0:b - a, P] = np.clip(erl[a:b] - (gbase + gl) * GR, -2, 20)
        # wrap idxs: idx i lives at [i%16, i//16]; replicate to 128 partitions
        idxw = np.tile(idx16.reshape(-1, 16).T, (8, 1))
        per_core.append(dict(
            idx=np.ascontiguousarray(idxw),
            c6=np.ascontiguousarray(_bf16(c6p)),
            jl=np.ascontiguousarray(_bf16(jlp)),
            cnt=np.ascontiguousarray(np.tile(cnt32, (128, 1))),
        ))

    # weight tiles [12][128, 256] f32: 0..5 = +W[k][fh], 6..11 = -W[k][fh]
    weight = np.asarray(weight, np.float32)
    wt = np.empty((12, 128, C), np.float32)
    for fh in range(2):
        for k in range(K1):
            wt[fh * 3 + k] = weight[k][fh * 128:(fh + 1) * 128]
            wt[6 + fh * 3 + k] = -weight[k][fh * 128:(fh + 1) * 128]
    wsb = np.ascontiguousarray(_bf16(wt.transpose(1, 0, 2).reshape(128, 12 * C)))

    biasr = np.ascontiguousarray(np.tile(np.asarray(bias, np.float32), (128, 1)))
    # 14-wide one-hot pattern per pair position: column j of pair r is j
    mdw = np.ascontiguousarray(_bf16(np.tile(
        (np.arange(npc_max * GR) % GR).astype(np.float32), (128, 1))))

    return dict(tot_blk=tot_blk, npair=npair, ncall=ncall, calls=calls,
                pairs_by_batch=pairs_by_batch, nbt_max=nbt_max,
                npbt_max=npbt_max, npc_max=npc_max, per_core=per_core,
                wsb=wsb, biasr=biasr, mdw=mdw)


def _final_mm_list():
    """(target, q, s, wtile): target 0=real 1=imag; q = P quarter; s = slot."""
    mms = []
    for tgt in range(2):
        for fh in range(2):
            for k in range(K1):
                if tgt == 0:
                    mms.append((0, fh, k, fh * 3 + k))               # +W  Lr@Xr
                    mms.append((0, 2 + fh, 3 + k, 6 + fh * 3 + k))   # -W  Li@Xi
                else:
                    mms.append((1, fh, 3 + k, fh * 3 + k))           # +W  Li@Xr
                    mms.append((1, 2 + fh, k, fh * 3 + k))           # +W  Lr@Xi
    return mms


def _build(nc, prep, repeat=1):
    import concourse.mybir as mybir
    from concourse.tile import TileContext

    f32 = mybir.dt.float32
    bf16 = mybir.dt.bfloat16
    i16 = mybir.dt.int16
    i32 = mybir.dt.int32
    tot_blk = prep["tot_blk"]
    npair = prep["npair"]
    ncall = prep["ncall"]
    calls = prep["calls"]
    pairs_by_batch = prep["pairs_by_batch"]
    nbt_max = prep["nbt_max"]
    npbt_max = prep["npbt_max"]
    npc_max = prep["npc_max"]

    xcat = nc.dram_tensor("xcat", [N, 512], bf16, kind="ExternalInput")
    idx_d = nc.dram_tensor("idx", [128, tot_blk * 8], i16, kind="ExternalInput")
    c6_d = nc.dram_tensor("c6", [128, npair * 6], bf16, kind="ExternalInput")
    jl_d = nc.dram_tensor("jl", [128, npair], bf16, kind="ExternalInput")
    cnt_d = nc.dram_tensor("cnt", [128, ncall], i32, kind="ExternalInput")
    w_d = nc.dram_tensor("wt", [128, 12 * C], bf16, kind="ExternalInput")
    bias_d = nc.dram_tensor("biasr", [128, C], f32, kind="ExternalInput")
    md_d = nc.dram_tensor("mdw", [128, npc_max * GR], bf16,
                          kind="ExternalInput")
    or_d = nc.dram_tensor("out_r", [NB * ROWS_PB, C], bf16, kind="ExternalOutput")
    oi_d = nc.dram_tensor("out_i", [NB * ROWS_PB, C], bf16, kind="ExternalOutput")

    mms = _final_mm_list()

    import contextlib

    with TileContext(nc) as tc:
        with tc.tile_pool(name="const", bufs=1) as cpool, \
             tc.tile_pool(name="g", bufs=GT_BUFS) as gpool, \
             tc.tile_pool(name="v", bufs=2) as vpool, \
             tc.tile_pool(name="ohp", bufs=2) as ohpool, \
             tc.tile_pool(name="pb", bufs=2) as pbpool, \
             tc.tile_pool(name="os", bufs=3) as ospool, \
             tc.tile_pool(name="ps", bufs=3, space="PSUM") as pspool, \
             tc.tile_pool(name="po", bufs=2, space="PSUM") as popool:

            idx_t = cpool.tile([128, tot_blk * 8], i16)
            c6_t = cpool.tile([128, npair * 6], bf16)
            jl_t = cpool.tile([128, npair], bf16)
            cnt_t = cpool.tile([128, ncall], i32)
            w_t = cpool.tile([128, 12 * C], bf16)
            bias_t = cpool.tile([128, C], f32)
            md_t = cpool.tile([128, npc_max * GR], bf16)
            for dst, src in [(idx_t, idx_d), (c6_t, c6_d), (jl_t, jl_d),
                             (cnt_t, cnt_d), (w_t, w_d), (bias_t, bias_d),
                             (md_t, md_d)]:
                nc.sync.dma_start(dst[:], src[:])

            rep_cm = tc.For_i(0, repeat, 1) if repeat > 1 else contextlib.nullcontext()
            with rep_cm:
              qn = 0
              for bt in range(NB):
                  c0, c1 = calls[2 * bt], calls[2 * bt + 1]
                  nb0 = c0["nbh"]
                  gt = gpool.tile([128, nbt_max * 512], bf16, tag="g")
                  if USE_COUNT_REG and bt < GT_BUFS:
                      # first touch of each ring buffer: idx=-1 pad lanes are
                      # never written by the gather, so clear once (later
                      # batches inherit finite stale data, zeroed by V)
                      nc.vector.memset(gt[:], 0.0)
                  for j, cl in enumerate((c0, c1)):
                      src = xcat[:] if cl["h"] == 0 else xcat[HALF:, :]
                      off = 0 if j == 0 else nb0
                      for ci_s, k0, nb_s in cl["subs"]:
                          if USE_COUNT_REG:
                              reg = nc.gpsimd.value_load(
                                  cnt_t[0:1, ci_s:ci_s + 1])
                          else:
                              reg = nb_s * 128
                          if ABLATE != "nogather":
                              nc.gpsimd.dma_gather(
                                  gt[:, (off + k0) * 512:(off + k0 + nb_s) * 512]
                                    .rearrange("p (b e) -> p b e", e=512),
                                  src,
                                  idx_t[:, (cl["b0"] + k0) * 8:
                                           (cl["b0"] + k0 + nb_s) * 8],
                                  nb_s * 128, reg, 512,
                                  queue_num=qn,
                              )
                              qn = (qn + 1) % NQ
                  if ABLATE == "nocompute":
                      o_r = ospool.tile([128, C], bf16, tag="or")
                      o_i = ospool.tile([128, C], bf16, tag="oi")
                      nc.scalar.copy(o_r[:ROWS_PB, :], bias_t[:ROWS_PB, :])
                      nc.scalar.copy(o_i[:ROWS_PB, :], bias_t[:ROWS_PB, :])
                      nc.sync.dma_start(or_d[bt * ROWS_PB:(bt + 1) * ROWS_PB, :],
                                        o_r[:ROWS_PB, :])
                      nc.sync.dma_start(oi_d[bt * ROWS_PB:(bt + 1) * ROWS_PB, :],
                                        o_i[:ROWS_PB, :])
                      continue
                  # one-hot V for every (block, group) pair of the batch:
                  # 14-wide one-hot (is_equal) then one broadcast multiply
                  # expands x6 slots with the edge vals
                  vt = vpool.tile([128, npbt_max * MCOLS], bf16, tag="v")
                  oh = ohpool.tile([128, npbt_max * GR], bf16, tag="oh")
                  voff = 0
                  for cl in (c0, c1):
                      np_ = len(cl["prs"])
                      if np_ == 0:
                          continue
                      ohsl = oh[:, voff * GR:(voff + np_) * GR]
                      nc.vector.tensor_tensor(
                          ohsl.rearrange("p (r j) -> p r j", j=GR),
                          md_t[:, :np_ * GR]
                              .rearrange("p (r j) -> p r j", j=GR),
                          jl_t[:, cl["p0"]:cl["p0"] + np_]
                              .unsqueeze(2).broadcast_to((128, np_, GR)),
                          mybir.AluOpType.is_equal)
                      nc.vector.tensor_tensor(
                          vt[:, voff * MCOLS:(voff + np_) * MCOLS]
                              .rearrange("p (r s j) -> p r s j", s=6, j=GR),
                          ohsl.rearrange("p (r j) -> p r j", j=GR)
                              .unsqueeze(2).broadcast_to((128, np_, 6, GR)),
                          c6_t[:, cl["p0"] * 6:(cl["p0"] + np_) * 6]
                              .rearrange("p (r s) -> p r s", s=6)
                              .unsqueeze(3).broadcast_to((128, np_, 6, GR)),
                          mybir.AluOpType.mult)
                      voff += np_
                  # per-group spmm accumulation (feature-major in PSUM)
                  pbuf = pbpool.tile([128, 24 * ROWS_PB], bf16, tag="pb")
                  pb5 = pbuf[:].rearrange("p (q s g j) -> p q s g j",
                                          q=4, s=6, g=GPB)
                  for gl in range(GPB):
                      prs = pairs_by_batch[bt][gl]
                      if not prs:
                          nc.vector.memset(pb5[:, :, :, gl, :], 0.0)
                          continue
                      p_t = pspool.tile([128, 512], f32, tag="p")
                      for q in range(4):
                          for i, (bl, vc) in enumerate(prs):
                              nc.tensor.matmul(
                                  p_t[:, q * 128:q * 128 + MCOLS],
                                  gt[:, bl * 512 + q * 128:bl * 512 + (q + 1) * 128],
                                  vt[:, vc * MCOLS:(vc + 1) * MCOLS],
                                  start=(i == 0), stop=(i == len(prs) - 1))
                      nc.scalar.copy(
                          pb5[:, :, :, gl, :],
                          p_t[:].rearrange("p (q x) -> p q x", x=128)[:, :, :MCOLS])
                  # final matmuls for this batch (f32r: 1 cycle/row at 256 cols)
                  po = popool.tile([128, 512], f32, tag="po")
                  nmm = {0: 0, 1: 0}
                  for tgt, q, s, wi in mms:
                      plane = q * 6 + s
                      nc.tensor.matmul(
                          po[:ROWS_PB, tgt * C:(tgt + 1) * C],
                          pbuf[:, plane * ROWS_PB:(plane + 1) * ROWS_PB],
                          w_t[:, wi * C:(wi + 1) * C],
                          start=(nmm[tgt] == 0), stop=(nmm[tgt] == 11))
                      nmm[tgt] += 1
                  o_r = ospool.tile([128, C], bf16, tag="or")
                  o_i = ospool.tile([128, C], bf16, tag="oi")
                  nc.vector.tensor_tensor(o_r[:ROWS_PB, :], po[:ROWS_PB, 0:C],
                                          bias_t[:ROWS_PB, :], mybir.AluOpType.add)
                  nc.vector.tensor_tensor(o_i[:ROWS_PB, :], po[:ROWS_PB, C:2 * C],
                                          bias_t[:ROWS_PB, :], mybir.AluOpType.add)
                  nc.sync.dma_start(or_d[bt * ROWS_PB:(bt + 1) * ROWS_PB, :],
                                    o_r[:ROWS_PB, :])
                  nc.sync.dma_start(oi_d[bt * ROWS_PB:(bt + 1) * ROWS_PB, :],
                                    o_i[:ROWS_PB, :])


def _make_nc(prep, repeat=1):
    import concourse.bacc as bacc
    nc = bacc.Bacc("TRN2", target_bir_lowering=False, debug=False,
                   num_swdge_queues=NQ)
    _build(nc, prep, repeat=repeat)
    nc.compile()
    return nc


def _in_maps(prep, X_real, X_imag):
    xcat = _bf16(np.concatenate(
        [np.asarray(X_real, np.float32), np.asarray(X_imag, np.float32)],
        axis=1))
    maps = []
    for c in range(CORES):
        pc = prep["per_core"][c]
        maps.append({
            "xcat": xcat, "idx": pc["idx"], "c6": pc["c6"], "jl": pc["jl"],
            "cnt": pc["cnt"], "wt": prep["wsb"], "biasr": prep["biasr"],
            "mdw": prep["mdw"],
        })
    return maps


def kernel(X_real, X_imag, L_real_vals, L_imag_vals, weight, bias, rows, cols):
    from concourse.bass_utils import run_bass_kernel_spmd

    prep = _preprocess(rows, cols, L_real_vals, L_imag_vals, weight, bias)
    nc = _make_nc(prep)
    res = run_bass_kernel_spmd(nc, _in_maps(prep, X_real, X_imag),
                               core_ids=list(range(CORES)))
    out_r = np.concatenate([res.results[c]["out_r"][:RPC] for c in range(CORES)], 0)
    out_i = np.concatenate([res.results[c]["out_i"][:RPC] for c in range(CORES)], 0)
    return out_r.astype(np.float32), out_i.astype(np.float32)
